# revision 5
# baseline (speedup 1.0000x reference)
"""Trainium2 Bass kernel for nn_CapsuleBase (gnn_message_passing).

Computes, across 8 NeuronCores (data-parallel over the entity dim):
    x       = tanh(init_embed @ W_pca + b_pca)        [200000, 4, 128]
    sub_emb = x[sub]                                  [4096, 512]
    rel_emb = tile(init_rel[rel], (1, 4))             [4096, 512]
    mi_loss = HSIC dependence loss over sub_emb       scalar

Sharding: init_embed rows are split 25000/core (padded to 25088 = 49*512).
The host pre-transposes the embedding shard to embT [128, rows] so that each
128-column slice is directly the lhsT operand of the PE matmul (out =
lhsT.T @ rhs) -- no on-device transposes needed.  The bias is folded in with
a K=1 ones-vector matmul accumulating into the same PSUM tile, so the whole
chunk epilogue is a single ScalarE Tanh from PSUM to SBUF.

sub_emb is recomputed from host-gathered init_embed[sub] rows (identical
arithmetic to the x path, so it matches x[sub] bitwise); each core handles
512 of the 4096 batch rows and also accumulates the 4 per-factor [128,128]
HSIC gram matrices on-device.  The host sums the 8 partial grams and
finishes the tiny 4x4 HSIC reduction.  rel_emb is a pure input gather done
on the host.
"""

import sys

if "/opt/trn_rl_repo" not in sys.path:
    sys.path.insert(0, "/opt/trn_rl_repo")

import numpy as np

N_CORES = 8
N_ENT = 200000
INIT_DIM = 128
K_FACTORS = 4
GCN_DIM = 128
NOUT = K_FACTORS * GCN_DIM  # 512
BATCH = 4096
ROWS = N_ENT // N_CORES  # 25000
SLAB = 512  # rows processed per DMA load (4 chunks of 128)
N_SLABS = (ROWS + SLAB - 1) // SLAB  # 49
PAD_ROWS = N_SLABS * SLAB  # 25088
SUB_ROWS = BATCH // N_CORES  # 512

_program_cache = {}


def _build_program(n_slabs=N_SLABS, use_f32r=True):
    import concourse.bass as bass  # noqa: F401
    import concourse.tile as tile
    from concourse import bacc, mybir

    f32 = mybir.dt.float32
    f32r = mybir.dt.float32r
    mdt = f32r if use_f32r else f32
    Tanh = mybir.ActivationFunctionType.Tanh

    pad_rows = n_slabs * SLAB

    nc = bacc.Bacc(
        "TRN2", target_bir_lowering=False, debug=False, num_devices=N_CORES
    )
    embT = nc.dram_tensor("embT", [INIT_DIM, pad_rows], mdt, kind="ExternalInput")
    subT = nc.dram_tensor("subT", [INIT_DIM, SUB_ROWS], mdt, kind="ExternalInput")
    w = nc.dram_tensor("w", [INIT_DIM, NOUT], mdt, kind="ExternalInput")
    biasrow = nc.dram_tensor("biasrow", [1, NOUT], mdt, kind="ExternalInput")
    ones = nc.dram_tensor("ones", [1, INIT_DIM], mdt, kind="ExternalInput")
    x_out = nc.dram_tensor("x_out", [pad_rows, NOUT], f32, kind="ExternalOutput")
    sub_out = nc.dram_tensor("sub_out", [SUB_ROWS, NOUT], f32, kind="ExternalOutput")
    gram_out = nc.dram_tensor(
        "gram_out", [K_FACTORS * GCN_DIM, GCN_DIM], f32, kind="ExternalOutput"
    )

    with tile.TileContext(nc) as tc:
        with (
            tc.tile_pool(name="const", bufs=1) as const_pool,
            tc.tile_pool(name="eslab", bufs=4) as e_pool,
            tc.tile_pool(name="xsb", bufs=4) as x_pool,
            tc.tile_pool(name="xps", bufs=4, space="PSUM") as xps_pool,
        ):
            w_sb = const_pool.tile([INIT_DIM, NOUT], mdt)
            nc.sync.dma_start(w_sb[:], w[:])
            bias_sb = const_pool.tile([1, NOUT], mdt)
            nc.sync.dma_start(bias_sb[:], biasrow[:])
            ones_sb = const_pool.tile([1, INIT_DIM], mdt)
            nc.sync.dma_start(ones_sb[:], ones[:])

            def chunk(lhs_slice, out_dram, out_row0, tanh_sbuf_tiles):
                """One 128-row chunk: matmul + bias + tanh + store."""
                ps = xps_pool.tile([128, NOUT], f32, tag="xps")
                nc.tensor.matmul(
                    ps[:], lhs_slice, w_sb[:],
                    start=True, stop=False,
                )
                nc.tensor.matmul(
                    ps[:], ones_sb[:], bias_sb[:],
                    start=False, stop=True,
                )
                t = x_pool.tile([128, NOUT], f32, tag="xsb")
                nc.scalar.activation(t[:], ps[:], Tanh)
                nc.sync.dma_start(out_dram[out_row0 : out_row0 + 128, :], t[:])
                if tanh_sbuf_tiles is not None:
                    tanh_sbuf_tiles.append(t)

            # ---- sub path: 512 batch rows + partial HSIC grams ----
            with tc.tile_pool(name="gram", bufs=1, space="PSUM") as gram_pool:
                gram_ps = [
                    gram_pool.tile([GCN_DIM, GCN_DIM], f32, name=f"gram{_k}")
                    for _k in range(K_FACTORS)
                ]
                s_slab = e_pool.tile([INIT_DIM, SLAB], mdt, tag="eslab")
                nc.sync.dma_start(s_slab[:], subT[:])
                s_tiles = []
                for c in range(SUB_ROWS // 128):
                    chunk(s_slab[:, c * 128 : (c + 1) * 128], sub_out, c * 128, s_tiles)
                for c, t in enumerate(s_tiles):
                    first = c == 0
                    last = c == len(s_tiles) - 1
                    for k in range(K_FACTORS):
                        sl = t[:, k * GCN_DIM : (k + 1) * GCN_DIM]
                        nc.tensor.matmul(
                            gram_ps[k][:], sl, sl, start=first, stop=last
                        )
                for k in range(K_FACTORS):
                    g_sb = x_pool.tile([GCN_DIM, GCN_DIM], f32, tag="gram_sb")
                    nc.vector.tensor_copy(g_sb[:], gram_ps[k][:])
                    nc.sync.dma_start(
                        gram_out[k * GCN_DIM : (k + 1) * GCN_DIM, :], g_sb[:]
                    )

            # ---- main path: entity embedding transform ----
            for s in range(n_slabs):
                slab = e_pool.tile([INIT_DIM, SLAB], mdt, tag="eslab")
                nc.sync.dma_start(slab[:], embT[:, s * SLAB : (s + 1) * SLAB])
                for c in range(SLAB // 128):
                    chunk(
                        slab[:, c * 128 : (c + 1) * 128],
                        x_out,
                        s * SLAB + c * 128,
                        None,
                    )

    nc.compile()
    return nc


def _get_program():
    key = (N_SLABS, True)
    if key not in _program_cache:
        _program_cache[key] = _build_program()
    return _program_cache[key]


def _prep_in_maps(init_embed, W_pca, b_pca, sub):
    init_embed = np.ascontiguousarray(init_embed, dtype=np.float32)
    embT = init_embed.T  # [128, 200000] view
    sub = np.asarray(sub)
    gathered_T = np.ascontiguousarray(init_embed[sub].T)  # [128, 4096]
    w = np.ascontiguousarray(W_pca, dtype=np.float32)
    biasrow = np.ascontiguousarray(b_pca, dtype=np.float32).reshape(1, NOUT)
    ones = np.ones((1, INIT_DIM), dtype=np.float32)
    in_maps = []
    for m in range(N_CORES):
        shard = np.zeros((INIT_DIM, PAD_ROWS), dtype=np.float32)
        shard[:, :ROWS] = embT[:, m * ROWS : (m + 1) * ROWS]
        in_maps.append(
            {
                "embT": shard,
                "subT": np.ascontiguousarray(
                    gathered_T[:, m * SUB_ROWS : (m + 1) * SUB_ROWS]
                ),
                "w": w,
                "biasrow": biasrow,
                "ones": ones,
            }
        )
    return in_maps


def _assemble(results, init_rel, rel):
    x = np.concatenate(
        [results[m]["x_out"][:ROWS] for m in range(N_CORES)], axis=0
    ).reshape(N_ENT, K_FACTORS, GCN_DIM)
    sub_emb = np.concatenate(
        [results[m]["sub_out"] for m in range(N_CORES)], axis=0
    )
    gram = np.zeros((K_FACTORS, GCN_DIM, GCN_DIM), dtype=np.float64)
    for m in range(N_CORES):
        gram += results[m]["gram_out"].reshape(K_FACTORS, GCN_DIM, GCN_DIM)

    init_rel = np.asarray(init_rel, dtype=np.float32)
    rel = np.asarray(rel)
    rel_emb = np.tile(init_rel[rel], (1, K_FACTORS))

    n = GCN_DIM
    hH = np.eye(n) - np.ones((n, n)) / n
    M = np.einsum("de,kef->kdf", hH, gram)
    G = np.einsum("idf,jfd->ij", M, M)
    mi_loss = np.float32((G.sum() - np.trace(G)) / 2.0)
    return sub_emb, rel_emb, x, mi_loss


def _run(inputs, trace=False):
    from concourse.bass_utils import run_bass_kernel_spmd

    nc = _get_program()
    in_maps = _prep_in_maps(
        inputs["init_embed"], inputs["W_pca"], inputs["b_pca"], inputs["sub"]
    )
    if trace:
        _install_ntff_hook()
    res = run_bass_kernel_spmd(nc, in_maps, list(range(N_CORES)), trace=trace)
    out = _assemble(res.results, inputs["init_rel"], inputs["rel"])
    return out, res


def kernel(**inputs):
    out, _ = _run(inputs, trace=False)
    return out


def _install_ntff_hook():
    """The agent image's antenv lacks axon_hooks; shim it so trace=True
    can capture an NTFF profile (used by test.py only)."""
    import types

    if "antenv.axon_hooks" in sys.modules:
        return
    import antenv
    from trn_agent_boot.trn_boot import _ntff_profile_via_ctypes

    mod = types.ModuleType("antenv.axon_hooks")
    mod._hook = _ntff_profile_via_ctypes("/opt/axon/libaxon_pjrt.so")
    mod.set_axon_ntff_profile_hook = lambda h: setattr(mod, "_hook", h)
    mod.get_axon_ntff_profile_hook = lambda: mod._hook
    sys.modules["antenv.axon_hooks"] = mod
    antenv.axon_hooks = mod


# revision 10
# speedup vs baseline: 1.3269x; 1.3269x over previous
"""Trainium2 Bass kernel for nn_CapsuleBase (gnn_message_passing).

Computes, across 8 NeuronCores (data-parallel over the entity dim):
    x       = tanh(init_embed @ W_pca + b_pca)        [200000, 4, 128]
    sub_emb = x[sub]                                  [4096, 512]
    rel_emb = tile(init_rel[rel], (1, 4))             [4096, 512]
    mi_loss = HSIC dependence loss over sub_emb       scalar

Sharding: init_embed rows are split 25000/core (padded to 25088 = 49*512).
The host pre-transposes the embedding shard to embT [128, rows] so that each
128-column slice is directly the lhsT operand of the PE matmul (out =
lhsT.T @ rhs) -- no on-device transposes needed.  The bias is folded in with
a K=1 ones-vector matmul accumulating into the same PSUM tile, so the whole
chunk epilogue is a single ScalarE Tanh from PSUM to SBUF.

sub_emb is recomputed from host-gathered init_embed[sub] rows (identical
arithmetic to the x path, so it matches x[sub] bitwise); each core handles
512 of the 4096 batch rows and also accumulates the 4 per-factor [128,128]
HSIC gram matrices on-device.  The host sums the 8 partial grams and
finishes the tiny 4x4 HSIC reduction.  rel_emb is a pure input gather done
on the host.
"""

import sys

if "/opt/trn_rl_repo" not in sys.path:
    sys.path.insert(0, "/opt/trn_rl_repo")

import numpy as np

N_CORES = 8
N_ENT = 200000
INIT_DIM = 128
K_FACTORS = 4
GCN_DIM = 128
NOUT = K_FACTORS * GCN_DIM  # 512
BATCH = 4096
ROWS = N_ENT // N_CORES  # 25000
SLAB = 512  # rows processed per DMA load (4 chunks of 128)
N_SLABS = (ROWS + SLAB - 1) // SLAB  # 49
PAD_ROWS = N_SLABS * SLAB  # 25088
SUB_ROWS = BATCH // N_CORES  # 512

_program_cache = {}


def _build_program(n_slabs=N_SLABS, use_f32r=True):
    import concourse.bass as bass  # noqa: F401
    import concourse.tile as tile
    from concourse import bacc, mybir

    f32 = mybir.dt.float32
    f32r = mybir.dt.float32r
    mdt = f32r if use_f32r else f32
    Tanh = mybir.ActivationFunctionType.Tanh

    pad_rows = n_slabs * SLAB

    nc = bacc.Bacc(
        "TRN2", target_bir_lowering=False, debug=False, num_devices=N_CORES
    )
    embT = nc.dram_tensor("embT", [INIT_DIM, pad_rows], mdt, kind="ExternalInput")
    subT = nc.dram_tensor("subT", [INIT_DIM, SUB_ROWS], mdt, kind="ExternalInput")
    w = nc.dram_tensor("w", [INIT_DIM, NOUT], mdt, kind="ExternalInput")
    biasrow = nc.dram_tensor("biasrow", [1, NOUT], mdt, kind="ExternalInput")
    biascol = nc.dram_tensor(
        "biascol", [GCN_DIM, K_FACTORS], f32, kind="ExternalInput"
    )
    ones = nc.dram_tensor("ones", [1, INIT_DIM], mdt, kind="ExternalInput")
    xT_out = nc.dram_tensor("xT_out", [NOUT, pad_rows], f32, kind="ExternalOutput")
    sub_out = nc.dram_tensor("sub_out", [SUB_ROWS, NOUT], f32, kind="ExternalOutput")
    gram_out = nc.dram_tensor(
        "gram_out", [K_FACTORS * GCN_DIM, GCN_DIM], f32, kind="ExternalOutput"
    )

    with tile.TileContext(nc) as tc:
        with (
            tc.tile_pool(name="const", bufs=1) as const_pool,
            tc.tile_pool(name="eslab", bufs=4) as e_pool,
            tc.tile_pool(name="xsb", bufs=4) as x_pool,
            tc.tile_pool(name="xps", bufs=4, space="PSUM") as xps_pool,
        ):
            w_sb = const_pool.tile([INIT_DIM, NOUT], mdt)
            nc.sync.dma_start(w_sb[:], w[:])
            bias_sb = const_pool.tile([1, NOUT], mdt)
            nc.sync.dma_start(bias_sb[:], biasrow[:])
            biascol_sb = const_pool.tile([GCN_DIM, K_FACTORS], f32)
            nc.sync.dma_start(biascol_sb[:], biascol[:])
            ones_sb = const_pool.tile([1, INIT_DIM], mdt)
            nc.sync.dma_start(ones_sb[:], ones[:])

            def chunk(lhs_slice, out_dram, out_row0, tanh_sbuf_tiles):
                """One 128-row chunk: matmul + bias + tanh + store."""
                ps = xps_pool.tile([128, NOUT], f32, tag="xps")
                nc.tensor.matmul(
                    ps[:], lhs_slice, w_sb[:],
                    start=True, stop=False,
                )
                nc.tensor.matmul(
                    ps[:], ones_sb[:], bias_sb[:],
                    start=False, stop=True,
                )
                t = x_pool.tile([128, NOUT], f32, tag="xsb")
                nc.scalar.activation(t[:], ps[:], Tanh)
                nc.sync.dma_start(out_dram[out_row0 : out_row0 + 128, :], t[:])
                if tanh_sbuf_tiles is not None:
                    tanh_sbuf_tiles.append(t)

            # ---- sub path: 512 batch rows + partial HSIC grams ----
            with tc.tile_pool(name="gram", bufs=1, space="PSUM") as gram_pool:
                gram_ps = [
                    gram_pool.tile([GCN_DIM, GCN_DIM], f32, name=f"gram{_k}")
                    for _k in range(K_FACTORS)
                ]
                s_slab = e_pool.tile([INIT_DIM, SLAB], mdt, tag="eslab")
                nc.sync.dma_start(s_slab[:], subT[:])
                s_tiles = []
                for c in range(SUB_ROWS // 128):
                    chunk(s_slab[:, c * 128 : (c + 1) * 128], sub_out, c * 128, s_tiles)
                for c, t in enumerate(s_tiles):
                    first = c == 0
                    last = c == len(s_tiles) - 1
                    for k in range(K_FACTORS):
                        sl = t[:, k * GCN_DIM : (k + 1) * GCN_DIM]
                        nc.tensor.matmul(
                            gram_ps[k][:], sl, sl, start=first, stop=last
                        )
                for k in range(K_FACTORS):
                    g_sb = x_pool.tile([GCN_DIM, GCN_DIM], f32, tag="gram_sb")
                    nc.vector.tensor_copy(g_sb[:], gram_ps[k][:])
                    nc.sync.dma_start(
                        gram_out[k * GCN_DIM : (k + 1) * GCN_DIM, :], g_sb[:]
                    )

            # ---- main path: entity embedding transform (transposed out) ----
            # out = lhsT.T @ rhs with lhsT = W[:, f-slice] (stationary, only 4
            # distinct weights) and rhs = embT slab (moving): one matmul per
            # 128-feature slice covers all 512 slab rows, and the bias is a
            # per-partition AP fused into the Tanh activation.
            for s in range(n_slabs):
                slab = e_pool.tile([INIT_DIM, SLAB], mdt, tag="eslab")
                nc.sync.dma_start(slab[:], embT[:, s * SLAB : (s + 1) * SLAB])
                for f in range(K_FACTORS):
                    ps = xps_pool.tile([128, SLAB], f32, tag="xps")
                    nc.tensor.matmul(
                        ps[:],
                        w_sb[:, f * GCN_DIM : (f + 1) * GCN_DIM],
                        slab[:],
                        start=True,
                        stop=True,
                    )
                    t = x_pool.tile([128, SLAB], f32, tag="xsb")
                    nc.scalar.activation(
                        t[:], ps[:], Tanh, bias=biascol_sb[:, f : f + 1]
                    )
                    nc.sync.dma_start(
                        xT_out[
                            f * GCN_DIM : (f + 1) * GCN_DIM,
                            s * SLAB : (s + 1) * SLAB,
                        ],
                        t[:],
                    )

    nc.compile()
    return nc


def _get_program():
    key = (N_SLABS, True)
    if key not in _program_cache:
        _program_cache[key] = _build_program()
    return _program_cache[key]


def _prep_in_maps(init_embed, W_pca, b_pca, sub):
    init_embed = np.ascontiguousarray(init_embed, dtype=np.float32)
    embT = init_embed.T  # [128, 200000] view
    sub = np.asarray(sub)
    gathered_T = np.ascontiguousarray(init_embed[sub].T)  # [128, 4096]
    w = np.ascontiguousarray(W_pca, dtype=np.float32)
    b = np.asarray(b_pca, dtype=np.float32)
    biasrow = np.ascontiguousarray(b).reshape(1, NOUT)
    biascol = np.ascontiguousarray(b.reshape(K_FACTORS, GCN_DIM).T)
    ones = np.ones((1, INIT_DIM), dtype=np.float32)
    in_maps = []
    for m in range(N_CORES):
        shard = np.zeros((INIT_DIM, PAD_ROWS), dtype=np.float32)
        shard[:, :ROWS] = embT[:, m * ROWS : (m + 1) * ROWS]
        in_maps.append(
            {
                "embT": shard,
                "subT": np.ascontiguousarray(
                    gathered_T[:, m * SUB_ROWS : (m + 1) * SUB_ROWS]
                ),
                "w": w,
                "biasrow": biasrow,
                "biascol": biascol,
                "ones": ones,
            }
        )
    return in_maps


def _assemble(results, init_rel, rel):
    x = np.empty((N_ENT, NOUT), dtype=np.float32)
    for m in range(N_CORES):
        xT = results[m]["xT_out"]  # [512, PAD_ROWS]
        x[m * ROWS : (m + 1) * ROWS] = xT[:, :ROWS].T
    x = x.reshape(N_ENT, K_FACTORS, GCN_DIM)
    sub_emb = np.concatenate(
        [results[m]["sub_out"] for m in range(N_CORES)], axis=0
    )
    gram = np.zeros((K_FACTORS, GCN_DIM, GCN_DIM), dtype=np.float64)
    for m in range(N_CORES):
        gram += results[m]["gram_out"].reshape(K_FACTORS, GCN_DIM, GCN_DIM)

    init_rel = np.asarray(init_rel, dtype=np.float32)
    rel = np.asarray(rel)
    rel_emb = np.tile(init_rel[rel], (1, K_FACTORS))

    n = GCN_DIM
    hH = np.eye(n) - np.ones((n, n)) / n
    M = np.einsum("de,kef->kdf", hH, gram)
    G = np.einsum("idf,jfd->ij", M, M)
    mi_loss = np.float32((G.sum() - np.trace(G)) / 2.0)
    return sub_emb, rel_emb, x, mi_loss


def _run(inputs, trace=False):
    from concourse.bass_utils import run_bass_kernel_spmd

    nc = _get_program()
    in_maps = _prep_in_maps(
        inputs["init_embed"], inputs["W_pca"], inputs["b_pca"], inputs["sub"]
    )
    if trace:
        _install_ntff_hook()
    res = run_bass_kernel_spmd(nc, in_maps, list(range(N_CORES)), trace=trace)
    out = _assemble(res.results, inputs["init_rel"], inputs["rel"])
    return out, res


def kernel(**inputs):
    out, _ = _run(inputs, trace=False)
    return out


def _install_ntff_hook():
    """The agent image's antenv lacks axon_hooks; shim it so trace=True
    can capture an NTFF profile (used by test.py only)."""
    import types

    if "antenv.axon_hooks" in sys.modules:
        return
    import antenv
    from trn_agent_boot.trn_boot import _ntff_profile_via_ctypes

    mod = types.ModuleType("antenv.axon_hooks")
    mod._hook = _ntff_profile_via_ctypes("/opt/axon/libaxon_pjrt.so")
    mod.set_axon_ntff_profile_hook = lambda h: setattr(mod, "_hook", h)
    mod.get_axon_ntff_profile_hook = lambda: mod._hook
    sys.modules["antenv.axon_hooks"] = mod
    antenv.axon_hooks = mod


# revision 14
# speedup vs baseline: 1.4459x; 1.0897x over previous
"""Trainium2 Bass kernel for nn_CapsuleBase (gnn_message_passing).

Computes, across 8 NeuronCores (data-parallel over the entity dim):
    x       = tanh(init_embed @ W_pca + b_pca)        [200000, 4, 128]
    sub_emb = x[sub]                                  [4096, 512]
    rel_emb = tile(init_rel[rel], (1, 4))             [4096, 512]
    mi_loss = HSIC dependence loss over sub_emb       scalar

Sharding: init_embed rows are split 25000/core (padded to 25088 = 49*512).
The host pre-transposes the embedding shard to embT [128, rows] so that each
128-column slice is directly the lhsT operand of the PE matmul (out =
lhsT.T @ rhs) -- no on-device transposes needed.  The bias is folded in with
a K=1 ones-vector matmul accumulating into the same PSUM tile, so the whole
chunk epilogue is a single ScalarE Tanh from PSUM to SBUF.

sub_emb is recomputed from host-gathered init_embed[sub] rows (identical
arithmetic to the x path, so it matches x[sub] bitwise); each core handles
512 of the 4096 batch rows and also accumulates the 4 per-factor [128,128]
HSIC gram matrices on-device.  The host sums the 8 partial grams and
finishes the tiny 4x4 HSIC reduction.  rel_emb is a pure input gather done
on the host.
"""

import sys

if "/opt/trn_rl_repo" not in sys.path:
    sys.path.insert(0, "/opt/trn_rl_repo")

import numpy as np

N_CORES = 8
N_ENT = 200000
INIT_DIM = 128
K_FACTORS = 4
GCN_DIM = 128
NOUT = K_FACTORS * GCN_DIM  # 512
BATCH = 4096
ROWS = N_ENT // N_CORES  # 25000
SLAB = 512  # rows processed per DMA load (4 chunks of 128)
N_SLABS = (ROWS + SLAB - 1) // SLAB  # 49
PAD_ROWS = N_SLABS * SLAB  # 25088
SUB_ROWS = BATCH // N_CORES  # 512

_program_cache = {}


def _build_program(n_slabs=N_SLABS, use_f32r=True):
    import concourse.bass as bass  # noqa: F401
    import concourse.tile as tile
    from concourse import bacc, mybir

    f32 = mybir.dt.float32
    f32r = mybir.dt.float32r
    mdt = f32r if use_f32r else f32
    Tanh = mybir.ActivationFunctionType.Tanh

    pad_rows = n_slabs * SLAB

    nc = bacc.Bacc(
        "TRN2", target_bir_lowering=False, debug=False, num_devices=N_CORES
    )
    embB = nc.dram_tensor(
        "embB", [n_slabs, INIT_DIM, SLAB], mdt, kind="ExternalInput"
    )
    subT = nc.dram_tensor("subT", [INIT_DIM, SUB_ROWS], mdt, kind="ExternalInput")
    w = nc.dram_tensor("w", [INIT_DIM, NOUT], mdt, kind="ExternalInput")
    biasrow = nc.dram_tensor("biasrow", [1, NOUT], mdt, kind="ExternalInput")
    biascol = nc.dram_tensor(
        "biascol", [GCN_DIM, K_FACTORS], f32, kind="ExternalInput"
    )
    ones = nc.dram_tensor("ones", [1, INIT_DIM], mdt, kind="ExternalInput")
    xB_out = nc.dram_tensor(
        "xB_out", [n_slabs, K_FACTORS, GCN_DIM, SLAB], f32, kind="ExternalOutput"
    )
    sub_out = nc.dram_tensor("sub_out", [SUB_ROWS, NOUT], f32, kind="ExternalOutput")
    gram_out = nc.dram_tensor(
        "gram_out", [K_FACTORS * GCN_DIM, GCN_DIM], f32, kind="ExternalOutput"
    )

    with tile.TileContext(nc) as tc:
        with (
            tc.tile_pool(name="const", bufs=1) as const_pool,
            tc.tile_pool(name="eslab", bufs=6) as e_pool,
            tc.tile_pool(name="xsb", bufs=8) as x_pool,
            tc.tile_pool(name="xps", bufs=4, space="PSUM") as xps_pool,
        ):
            w_sb = const_pool.tile([INIT_DIM, NOUT], mdt)
            nc.sync.dma_start(w_sb[:], w[:])
            bias_sb = const_pool.tile([1, NOUT], mdt)
            nc.sync.dma_start(bias_sb[:], biasrow[:])
            biascol_sb = const_pool.tile([GCN_DIM, K_FACTORS], f32)
            nc.sync.dma_start(biascol_sb[:], biascol[:])
            ones_sb = const_pool.tile([1, INIT_DIM], mdt)
            nc.sync.dma_start(ones_sb[:], ones[:])

            def chunk(lhs_slice, out_dram, out_row0, tanh_sbuf_tiles):
                """One 128-row chunk: matmul + bias + tanh + store."""
                ps = xps_pool.tile([128, NOUT], f32, tag="xps")
                nc.tensor.matmul(
                    ps[:], lhs_slice, w_sb[:],
                    start=True, stop=False,
                )
                nc.tensor.matmul(
                    ps[:], ones_sb[:], bias_sb[:],
                    start=False, stop=True,
                )
                t = x_pool.tile([128, NOUT], f32, tag="xsb")
                nc.scalar.activation(t[:], ps[:], Tanh)
                nc.sync.dma_start(out_dram[out_row0 : out_row0 + 128, :], t[:])
                if tanh_sbuf_tiles is not None:
                    tanh_sbuf_tiles.append(t)

            # ---- sub path: 512 batch rows + partial HSIC grams ----
            with tc.tile_pool(name="gram", bufs=1, space="PSUM") as gram_pool:
                gram_ps = [
                    gram_pool.tile([GCN_DIM, GCN_DIM], f32, name=f"gram{_k}")
                    for _k in range(K_FACTORS)
                ]
                s_slab = e_pool.tile([INIT_DIM, SLAB], mdt, tag="eslab")
                nc.gpsimd.dma_start(s_slab[:], subT[:])
                s_tiles = []
                for c in range(SUB_ROWS // 128):
                    chunk(s_slab[:, c * 128 : (c + 1) * 128], sub_out, c * 128, s_tiles)
                for c, t in enumerate(s_tiles):
                    first = c == 0
                    last = c == len(s_tiles) - 1
                    for k in range(K_FACTORS):
                        sl = t[:, k * GCN_DIM : (k + 1) * GCN_DIM]
                        nc.tensor.matmul(
                            gram_ps[k][:], sl, sl, start=first, stop=last
                        )
                for k in range(K_FACTORS):
                    g_sb = x_pool.tile([GCN_DIM, GCN_DIM], f32, tag="gram_sb")
                    nc.vector.tensor_copy(g_sb[:], gram_ps[k][:])
                    nc.sync.dma_start(
                        gram_out[k * GCN_DIM : (k + 1) * GCN_DIM, :], g_sb[:]
                    )

            # ---- main path: entity embedding transform (transposed out) ----
            # out = lhsT.T @ rhs with lhsT = W[:, f-slice] (stationary, only 4
            # distinct weights) and rhs = embT slab (moving): one matmul per
            # 128-feature slice covers all 512 slab rows, and the bias is a
            # per-partition AP fused into the Tanh activation.
            for s in range(n_slabs):
                slab = e_pool.tile([INIT_DIM, SLAB], mdt, tag="eslab")
                nc.gpsimd.dma_start(slab[:], embB[s])
                for f in range(K_FACTORS):
                    ps = xps_pool.tile([128, SLAB], f32, tag="xps")
                    nc.tensor.matmul(
                        ps[:],
                        w_sb[:, f * GCN_DIM : (f + 1) * GCN_DIM],
                        slab[:],
                        start=True,
                        stop=True,
                    )
                    t = x_pool.tile([128, SLAB], f32, tag="xsb")
                    nc.scalar.activation(
                        t[:], ps[:], Tanh, bias=biascol_sb[:, f : f + 1]
                    )
                    nc.sync.dma_start(xB_out[s, f], t[:])

    nc.compile()
    return nc


def _get_program():
    key = (N_SLABS, True)
    if key not in _program_cache:
        _program_cache[key] = _build_program()
    return _program_cache[key]


def _prep_in_maps(init_embed, W_pca, b_pca, sub):
    init_embed = np.ascontiguousarray(init_embed, dtype=np.float32)
    sub = np.asarray(sub)
    gathered_T = np.ascontiguousarray(init_embed[sub].T)  # [128, 4096]
    w = np.ascontiguousarray(W_pca, dtype=np.float32)
    b = np.asarray(b_pca, dtype=np.float32)
    biasrow = np.ascontiguousarray(b).reshape(1, NOUT)
    biascol = np.ascontiguousarray(b.reshape(K_FACTORS, GCN_DIM).T)
    ones = np.ones((1, INIT_DIM), dtype=np.float32)
    in_maps = []
    for m in range(N_CORES):
        pad = np.zeros((PAD_ROWS, INIT_DIM), dtype=np.float32)
        pad[:ROWS] = init_embed[m * ROWS : (m + 1) * ROWS]
        shard = np.ascontiguousarray(
            pad.reshape(N_SLABS, SLAB, INIT_DIM).transpose(0, 2, 1)
        )
        in_maps.append(
            {
                "embB": shard,
                "subT": np.ascontiguousarray(
                    gathered_T[:, m * SUB_ROWS : (m + 1) * SUB_ROWS]
                ),
                "w": w,
                "biasrow": biasrow,
                "biascol": biascol,
                "ones": ones,
            }
        )
    return in_maps


def _assemble(results, init_rel, rel):
    x = np.empty((N_ENT, NOUT), dtype=np.float32)
    for m in range(N_CORES):
        xB = results[m]["xB_out"]  # [N_SLABS, 4, 128, SLAB]
        xm = xB.transpose(0, 3, 1, 2).reshape(PAD_ROWS, NOUT)
        x[m * ROWS : (m + 1) * ROWS] = xm[:ROWS]
    x = x.reshape(N_ENT, K_FACTORS, GCN_DIM)
    sub_emb = np.concatenate(
        [results[m]["sub_out"] for m in range(N_CORES)], axis=0
    )
    gram = np.zeros((K_FACTORS, GCN_DIM, GCN_DIM), dtype=np.float64)
    for m in range(N_CORES):
        gram += results[m]["gram_out"].reshape(K_FACTORS, GCN_DIM, GCN_DIM)

    init_rel = np.asarray(init_rel, dtype=np.float32)
    rel = np.asarray(rel)
    rel_emb = np.tile(init_rel[rel], (1, K_FACTORS))

    n = GCN_DIM
    hH = np.eye(n) - np.ones((n, n)) / n
    M = np.einsum("de,kef->kdf", hH, gram)
    G = np.einsum("idf,jfd->ij", M, M)
    mi_loss = np.float32((G.sum() - np.trace(G)) / 2.0)
    return sub_emb, rel_emb, x, mi_loss


def _run(inputs, trace=False):
    from concourse.bass_utils import run_bass_kernel_spmd

    nc = _get_program()
    in_maps = _prep_in_maps(
        inputs["init_embed"], inputs["W_pca"], inputs["b_pca"], inputs["sub"]
    )
    if trace:
        _install_ntff_hook()
    res = run_bass_kernel_spmd(nc, in_maps, list(range(N_CORES)), trace=trace)
    out = _assemble(res.results, inputs["init_rel"], inputs["rel"])
    return out, res


def kernel(**inputs):
    out, _ = _run(inputs, trace=False)
    return out


def _install_ntff_hook():
    """The agent image's antenv lacks axon_hooks; shim it so trace=True
    can capture an NTFF profile (used by test.py only)."""
    import types

    if "antenv.axon_hooks" in sys.modules:
        return
    import antenv
    from trn_agent_boot.trn_boot import _ntff_profile_via_ctypes

    mod = types.ModuleType("antenv.axon_hooks")
    mod._hook = _ntff_profile_via_ctypes("/opt/axon/libaxon_pjrt.so")
    mod.set_axon_ntff_profile_hook = lambda h: setattr(mod, "_hook", h)
    mod.get_axon_ntff_profile_hook = lambda: mod._hook
    sys.modules["antenv.axon_hooks"] = mod
    antenv.axon_hooks = mod


# revision 17
# speedup vs baseline: 1.4821x; 1.0250x over previous
"""Trainium2 Bass kernel for nn_CapsuleBase (gnn_message_passing).

Computes, across 8 NeuronCores (data-parallel over the entity dim):
    x       = tanh(init_embed @ W_pca + b_pca)        [200000, 4, 128]
    sub_emb = x[sub]                                  [4096, 512]
    rel_emb = tile(init_rel[rel], (1, 4))             [4096, 512]
    mi_loss = HSIC dependence loss over sub_emb       scalar

Sharding: init_embed rows are split 25000/core (padded to 25088 = 49*512).
The host pre-transposes the embedding shard to embT [128, rows] so that each
128-column slice is directly the lhsT operand of the PE matmul (out =
lhsT.T @ rhs) -- no on-device transposes needed.  The bias is folded in with
a K=1 ones-vector matmul accumulating into the same PSUM tile, so the whole
chunk epilogue is a single ScalarE Tanh from PSUM to SBUF.

sub_emb is recomputed from host-gathered init_embed[sub] rows (identical
arithmetic to the x path, so it matches x[sub] bitwise); each core handles
512 of the 4096 batch rows and also accumulates the 4 per-factor [128,128]
HSIC gram matrices on-device.  The host sums the 8 partial grams and
finishes the tiny 4x4 HSIC reduction.  rel_emb is a pure input gather done
on the host.
"""

import sys

if "/opt/trn_rl_repo" not in sys.path:
    sys.path.insert(0, "/opt/trn_rl_repo")

import numpy as np

N_CORES = 8
N_ENT = 200000
INIT_DIM = 128
K_FACTORS = 4
GCN_DIM = 128
NOUT = K_FACTORS * GCN_DIM  # 512
BATCH = 4096
ROWS = N_ENT // N_CORES  # 25000
SLAB = 512  # rows processed per DMA load (4 chunks of 128)
N_SLABS = (ROWS + SLAB - 1) // SLAB  # 49
PAD_ROWS = N_SLABS * SLAB  # 25088
SUB_ROWS = BATCH // N_CORES  # 512

_program_cache = {}


def _build_program(n_slabs=N_SLABS, use_f32r=True):
    import concourse.bass as bass  # noqa: F401
    import concourse.tile as tile
    from concourse import bacc, mybir

    f32 = mybir.dt.float32
    f32r = mybir.dt.float32r
    mdt = f32r if use_f32r else f32
    Tanh = mybir.ActivationFunctionType.Tanh

    pad_rows = n_slabs * SLAB

    nc = bacc.Bacc(
        "TRN2", target_bir_lowering=False, debug=False, num_devices=N_CORES
    )
    embB = nc.dram_tensor(
        "embB", [n_slabs, INIT_DIM, SLAB], mdt, kind="ExternalInput"
    )
    subT = nc.dram_tensor("subT", [INIT_DIM, SUB_ROWS], mdt, kind="ExternalInput")
    w = nc.dram_tensor("w", [INIT_DIM, NOUT], mdt, kind="ExternalInput")
    biasrow = nc.dram_tensor("biasrow", [1, NOUT], mdt, kind="ExternalInput")
    biascol = nc.dram_tensor(
        "biascol", [GCN_DIM, K_FACTORS], f32, kind="ExternalInput"
    )
    ones = nc.dram_tensor("ones", [1, INIT_DIM], mdt, kind="ExternalInput")
    # x output, laid out [slab][d][f][c] so each slab's store is a single
    # 1MB DMA with 8KB-contiguous runs per partition (d = feature row within
    # a 128-block, f = factor block, c = row within slab).
    xP_out = nc.dram_tensor(
        "xP_out", [n_slabs, GCN_DIM, K_FACTORS, SLAB], f32, kind="ExternalOutput"
    )
    sub_out = nc.dram_tensor("sub_out", [SUB_ROWS, NOUT], f32, kind="ExternalOutput")
    gram_out = nc.dram_tensor(
        "gram_out", [K_FACTORS * GCN_DIM, GCN_DIM], f32, kind="ExternalOutput"
    )

    with tile.TileContext(nc) as tc:
        with (
            tc.tile_pool(name="const", bufs=1) as const_pool,
            tc.tile_pool(name="eslab", bufs=6) as e_pool,
            tc.tile_pool(name="xsb", bufs=6) as x_pool,
            tc.tile_pool(name="xps", bufs=4, space="PSUM") as xps_pool,
        ):
            w_sb = const_pool.tile([INIT_DIM, NOUT], mdt)
            nc.sync.dma_start(w_sb[:], w[:])
            bias_sb = const_pool.tile([1, NOUT], mdt)
            nc.sync.dma_start(bias_sb[:], biasrow[:])
            biascol_sb = const_pool.tile([GCN_DIM, K_FACTORS], f32)
            nc.sync.dma_start(biascol_sb[:], biascol[:])
            ones_sb = const_pool.tile([1, INIT_DIM], mdt)
            nc.sync.dma_start(ones_sb[:], ones[:])

            def chunk(lhs_slice, out_dram, out_row0, tanh_sbuf_tiles):
                """One 128-row chunk: matmul + bias + tanh + store."""
                ps = xps_pool.tile([128, NOUT], f32, tag="xps")
                nc.tensor.matmul(
                    ps[:], lhs_slice, w_sb[:],
                    start=True, stop=False,
                )
                nc.tensor.matmul(
                    ps[:], ones_sb[:], bias_sb[:],
                    start=False, stop=True,
                )
                t = x_pool.tile([128, NOUT], f32, tag="xsb")
                nc.scalar.activation(t[:], ps[:], Tanh)
                nc.sync.dma_start(out_dram[out_row0 : out_row0 + 128, :], t[:])
                if tanh_sbuf_tiles is not None:
                    tanh_sbuf_tiles.append(t)

            # ---- sub path: 512 batch rows + partial HSIC grams ----
            with tc.tile_pool(name="gram", bufs=1, space="PSUM") as gram_pool:
                gram_ps = [
                    gram_pool.tile([GCN_DIM, GCN_DIM], f32, name=f"gram{_k}")
                    for _k in range(K_FACTORS)
                ]
                s_slab = e_pool.tile([INIT_DIM, SLAB], mdt, tag="eslab")
                nc.gpsimd.dma_start(s_slab[:], subT[:])
                s_tiles = []
                for c in range(SUB_ROWS // 128):
                    chunk(s_slab[:, c * 128 : (c + 1) * 128], sub_out, c * 128, s_tiles)
                for c, t in enumerate(s_tiles):
                    first = c == 0
                    last = c == len(s_tiles) - 1
                    for k in range(K_FACTORS):
                        sl = t[:, k * GCN_DIM : (k + 1) * GCN_DIM]
                        nc.tensor.matmul(
                            gram_ps[k][:], sl, sl, start=first, stop=last
                        )
                for k in range(K_FACTORS):
                    g_sb = x_pool.tile([GCN_DIM, GCN_DIM], f32, tag="gram_sb")
                    nc.vector.tensor_copy(g_sb[:], gram_ps[k][:])
                    nc.sync.dma_start(
                        gram_out[k * GCN_DIM : (k + 1) * GCN_DIM, :], g_sb[:]
                    )

            # ---- main path: entity embedding transform (transposed out) ----
            # out = lhsT.T @ rhs with lhsT = W[:, f-slice] (stationary, only 4
            # distinct weights) and rhs = embT slab (moving): one matmul per
            # 128-feature slice covers all 512 slab rows, and the bias is a
            # per-partition AP fused into the Tanh activation.
            for s in range(n_slabs):
                slab = e_pool.tile([INIT_DIM, SLAB], mdt, tag="eslab")
                nc.gpsimd.dma_start(slab[:], embB[s])
                t = x_pool.tile([128, K_FACTORS * SLAB], f32, tag="xsb")
                for f in range(K_FACTORS):
                    ps = xps_pool.tile([128, SLAB], f32, tag="xps")
                    nc.tensor.matmul(
                        ps[:],
                        w_sb[:, f * GCN_DIM : (f + 1) * GCN_DIM],
                        slab[:],
                        start=True,
                        stop=True,
                    )
                    nc.scalar.activation(
                        t[:, f * SLAB : (f + 1) * SLAB],
                        ps[:],
                        Tanh,
                        bias=biascol_sb[:, f : f + 1],
                    )
                nc.sync.dma_start(xP_out[s], t[:])

    nc.compile()
    return nc


def _get_program():
    key = (N_SLABS, True)
    if key not in _program_cache:
        _program_cache[key] = _build_program()
    return _program_cache[key]


def _prep_in_maps(init_embed, W_pca, b_pca, sub):
    init_embed = np.ascontiguousarray(init_embed, dtype=np.float32)
    sub = np.asarray(sub)
    gathered_T = np.ascontiguousarray(init_embed[sub].T)  # [128, 4096]
    w = np.ascontiguousarray(W_pca, dtype=np.float32)
    b = np.asarray(b_pca, dtype=np.float32)
    biasrow = np.ascontiguousarray(b).reshape(1, NOUT)
    biascol = np.ascontiguousarray(b.reshape(K_FACTORS, GCN_DIM).T)
    ones = np.ones((1, INIT_DIM), dtype=np.float32)
    in_maps = []
    for m in range(N_CORES):
        pad = np.zeros((PAD_ROWS, INIT_DIM), dtype=np.float32)
        pad[:ROWS] = init_embed[m * ROWS : (m + 1) * ROWS]
        shard = np.ascontiguousarray(
            pad.reshape(N_SLABS, SLAB, INIT_DIM).transpose(0, 2, 1)
        )
        in_maps.append(
            {
                "embB": shard,
                "subT": np.ascontiguousarray(
                    gathered_T[:, m * SUB_ROWS : (m + 1) * SUB_ROWS]
                ),
                "w": w,
                "biasrow": biasrow,
                "biascol": biascol,
                "ones": ones,
            }
        )
    return in_maps


def _assemble(results, init_rel, rel):
    x = np.empty((N_ENT, NOUT), dtype=np.float32)
    for m in range(N_CORES):
        xP = results[m]["xP_out"]  # [N_SLABS, 128, 4, SLAB]
        xm = xP.transpose(0, 3, 2, 1).reshape(PAD_ROWS, NOUT)
        x[m * ROWS : (m + 1) * ROWS] = xm[:ROWS]
    x = x.reshape(N_ENT, K_FACTORS, GCN_DIM)
    sub_emb = np.concatenate(
        [results[m]["sub_out"] for m in range(N_CORES)], axis=0
    )
    gram = np.zeros((K_FACTORS, GCN_DIM, GCN_DIM), dtype=np.float64)
    for m in range(N_CORES):
        gram += results[m]["gram_out"].reshape(K_FACTORS, GCN_DIM, GCN_DIM)

    init_rel = np.asarray(init_rel, dtype=np.float32)
    rel = np.asarray(rel)
    rel_emb = np.tile(init_rel[rel], (1, K_FACTORS))

    n = GCN_DIM
    hH = np.eye(n) - np.ones((n, n)) / n
    M = np.einsum("de,kef->kdf", hH, gram)
    G = np.einsum("idf,jfd->ij", M, M)
    mi_loss = np.float32((G.sum() - np.trace(G)) / 2.0)
    return sub_emb, rel_emb, x, mi_loss


def _run(inputs, trace=False):
    from concourse.bass_utils import run_bass_kernel_spmd

    nc = _get_program()
    in_maps = _prep_in_maps(
        inputs["init_embed"], inputs["W_pca"], inputs["b_pca"], inputs["sub"]
    )
    if trace:
        _install_ntff_hook()
    res = run_bass_kernel_spmd(nc, in_maps, list(range(N_CORES)), trace=trace)
    out = _assemble(res.results, inputs["init_rel"], inputs["rel"])
    return out, res


def kernel(**inputs):
    out, _ = _run(inputs, trace=False)
    return out


def _install_ntff_hook():
    """The agent image's antenv lacks axon_hooks; shim it so trace=True
    can capture an NTFF profile (used by test.py only)."""
    import types

    if "antenv.axon_hooks" in sys.modules:
        return
    import antenv
    from trn_agent_boot.trn_boot import _ntff_profile_via_ctypes

    mod = types.ModuleType("antenv.axon_hooks")
    mod._hook = _ntff_profile_via_ctypes("/opt/axon/libaxon_pjrt.so")
    mod.set_axon_ntff_profile_hook = lambda h: setattr(mod, "_hook", h)
    mod.get_axon_ntff_profile_hook = lambda: mod._hook
    sys.modules["antenv.axon_hooks"] = mod
    antenv.axon_hooks = mod


# revision 19
# speedup vs baseline: 1.7646x; 1.1907x over previous
"""Trainium2 Bass kernel for nn_CapsuleBase (gnn_message_passing).

Computes, across 8 NeuronCores (data-parallel over the entity dim):
    x       = tanh(init_embed @ W_pca + b_pca)        [200000, 4, 128]
    sub_emb = x[sub]                                  [4096, 512]
    rel_emb = tile(init_rel[rel], (1, 4))             [4096, 512]
    mi_loss = HSIC dependence loss over sub_emb       scalar

Sharding: init_embed rows are split 25000/core (padded to 25088 = 49*512).
The host pre-transposes the embedding shard to embT [128, rows] so that each
128-column slice is directly the lhsT operand of the PE matmul (out =
lhsT.T @ rhs) -- no on-device transposes needed.  The bias is folded in with
a K=1 ones-vector matmul accumulating into the same PSUM tile, so the whole
chunk epilogue is a single ScalarE Tanh from PSUM to SBUF.

sub_emb is recomputed from host-gathered init_embed[sub] rows (identical
arithmetic to the x path, so it matches x[sub] bitwise); each core handles
512 of the 4096 batch rows and also accumulates the 4 per-factor [128,128]
HSIC gram matrices on-device.  The host sums the 8 partial grams and
finishes the tiny 4x4 HSIC reduction.  rel_emb is a pure input gather done
on the host.
"""

import sys

if "/opt/trn_rl_repo" not in sys.path:
    sys.path.insert(0, "/opt/trn_rl_repo")

import numpy as np

N_CORES = 8
N_ENT = 200000
INIT_DIM = 128
K_FACTORS = 4
GCN_DIM = 128
NOUT = K_FACTORS * GCN_DIM  # 512
BATCH = 4096
ROWS = N_ENT // N_CORES  # 25000
SLAB = 512  # rows processed per DMA load (4 chunks of 128)
N_SLABS = (ROWS + SLAB - 1) // SLAB  # 49
PAD_ROWS = N_SLABS * SLAB  # 25088
SUB_ROWS = BATCH // N_CORES  # 512

_program_cache = {}


def _build_program(n_slabs=N_SLABS, use_f32r=True):
    import concourse.bass as bass  # noqa: F401
    import concourse.tile as tile
    from concourse import bacc, mybir

    f32 = mybir.dt.float32
    f32r = mybir.dt.float32r
    mdt = f32r if use_f32r else f32
    Tanh = mybir.ActivationFunctionType.Tanh

    pad_rows = n_slabs * SLAB

    nc = bacc.Bacc(
        "TRN2", target_bir_lowering=False, debug=False, num_devices=N_CORES
    )
    embB = nc.dram_tensor(
        "embB", [n_slabs, INIT_DIM, SLAB], mdt, kind="ExternalInput"
    )
    subT = nc.dram_tensor("subT", [INIT_DIM, SUB_ROWS], mdt, kind="ExternalInput")
    w = nc.dram_tensor("w", [INIT_DIM, NOUT], mdt, kind="ExternalInput")
    biasrow = nc.dram_tensor("biasrow", [1, NOUT], mdt, kind="ExternalInput")
    biascol = nc.dram_tensor(
        "biascol", [GCN_DIM, K_FACTORS], f32, kind="ExternalInput"
    )
    ones = nc.dram_tensor("ones", [1, INIT_DIM], mdt, kind="ExternalInput")
    # x output, laid out [slab][d][f][c] so each slab's store is a single
    # 1MB DMA with 8KB-contiguous runs per partition (d = feature row within
    # a 128-block, f = factor block, c = row within slab).
    xP_out = nc.dram_tensor(
        "xP_out", [n_slabs, GCN_DIM, K_FACTORS, SLAB], f32, kind="ExternalOutput"
    )
    sub_out = nc.dram_tensor("sub_out", [SUB_ROWS, NOUT], f32, kind="ExternalOutput")
    gram_out = nc.dram_tensor(
        "gram_out", [K_FACTORS * GCN_DIM, GCN_DIM], f32, kind="ExternalOutput"
    )

    with tile.TileContext(nc) as tc:
        with (
            tc.tile_pool(name="const", bufs=1) as const_pool,
            tc.tile_pool(name="eslab", bufs=12) as e_pool,
            tc.tile_pool(name="xsb", bufs=6) as x_pool,
            tc.tile_pool(name="xps", bufs=4, space="PSUM") as xps_pool,
        ):
            w_sb = const_pool.tile([INIT_DIM, NOUT], mdt)
            nc.sync.dma_start(w_sb[:], w[:])
            bias_sb = const_pool.tile([1, NOUT], mdt)
            nc.sync.dma_start(bias_sb[:], biasrow[:])
            biascol_sb = const_pool.tile([GCN_DIM, K_FACTORS], f32)
            nc.sync.dma_start(biascol_sb[:], biascol[:])
            ones_sb = const_pool.tile([1, INIT_DIM], mdt)
            nc.sync.dma_start(ones_sb[:], ones[:])

            def chunk(lhs_slice, out_dram, out_row0, tanh_sbuf_tiles):
                """One 128-row chunk: matmul + bias + tanh + store."""
                ps = xps_pool.tile([128, NOUT], f32, tag="xps")
                nc.tensor.matmul(
                    ps[:], lhs_slice, w_sb[:],
                    start=True, stop=False,
                )
                nc.tensor.matmul(
                    ps[:], ones_sb[:], bias_sb[:],
                    start=False, stop=True,
                )
                t = x_pool.tile([128, NOUT], f32, tag="subsb", bufs=5)
                nc.scalar.activation(t[:], ps[:], Tanh)
                nc.sync.dma_start(out_dram[out_row0 : out_row0 + 128, :], t[:])
                if tanh_sbuf_tiles is not None:
                    tanh_sbuf_tiles.append(t)

            # ---- main path: entity embedding transform (transposed out) ----
            # out = lhsT.T @ rhs with lhsT = W[:, f-slice] (stationary, only 4
            # distinct weights) and rhs = embT slab (moving): one matmul per
            # 128-feature slice covers all 512 slab rows, and the bias is a
            # per-partition AP fused into the Tanh activation.
            def main_slab(s):
                slab = e_pool.tile([INIT_DIM, SLAB], mdt, tag="eslab", name="slab")
                nc.gpsimd.dma_start(slab[:], embB[s])
                t = x_pool.tile([128, K_FACTORS * SLAB], f32, tag="xsb", name="t")
                for f in range(K_FACTORS):
                    ps = xps_pool.tile([128, SLAB], f32, tag="xps", name="ps")
                    nc.tensor.matmul(
                        ps[:],
                        w_sb[:, f * GCN_DIM : (f + 1) * GCN_DIM],
                        slab[:],
                        start=True,
                        stop=True,
                    )
                    nc.scalar.activation(
                        t[:, f * SLAB : (f + 1) * SLAB],
                        ps[:],
                        Tanh,
                        bias=biascol_sb[:, f : f + 1],
                    )
                nc.sync.dma_start(xP_out[s], t[:])

            # A few main slabs first so PE and the store pipeline ramp
            # immediately; the small sub/gram work slots in mid-stream where
            # PE and DMA have slack.
            with tc.tile_pool(name="gram", bufs=1, space="PSUM") as gram_pool:
                gram_ps = [
                    gram_pool.tile([GCN_DIM, GCN_DIM], f32, name=f"gram{_k}")
                    for _k in range(K_FACTORS)
                ]
                sub_start = min(4, n_slabs)
                gram_at = min(12, n_slabs)
                for s in range(sub_start):
                    main_slab(s)

                # sub path: 512 batch rows, natural layout, ones-vector bias
                s_slab = const_pool.tile([INIT_DIM, SLAB], mdt)
                nc.gpsimd.dma_start(s_slab[:], subT[:])
                s_tiles = []
                for c in range(SUB_ROWS // 128):
                    chunk(s_slab[:, c * 128 : (c + 1) * 128], sub_out, c * 128, s_tiles)

                for s in range(sub_start, gram_at):
                    main_slab(s)

                for c, t in enumerate(s_tiles):
                    first = c == 0
                    last = c == len(s_tiles) - 1
                    for k in range(K_FACTORS):
                        sl = t[:, k * GCN_DIM : (k + 1) * GCN_DIM]
                        nc.tensor.matmul(
                            gram_ps[k][:], sl, sl, start=first, stop=last
                        )
                for k in range(K_FACTORS):
                    g_sb = x_pool.tile([GCN_DIM, GCN_DIM], f32, tag="gram_sb")
                    nc.vector.tensor_copy(g_sb[:], gram_ps[k][:])
                    nc.sync.dma_start(
                        gram_out[k * GCN_DIM : (k + 1) * GCN_DIM, :], g_sb[:]
                    )

                for s in range(gram_at, n_slabs):
                    main_slab(s)

    nc.compile()
    return nc


def _get_program():
    key = (N_SLABS, True)
    if key not in _program_cache:
        _program_cache[key] = _build_program()
    return _program_cache[key]


def _prep_in_maps(init_embed, W_pca, b_pca, sub):
    init_embed = np.ascontiguousarray(init_embed, dtype=np.float32)
    sub = np.asarray(sub)
    gathered_T = np.ascontiguousarray(init_embed[sub].T)  # [128, 4096]
    w = np.ascontiguousarray(W_pca, dtype=np.float32)
    b = np.asarray(b_pca, dtype=np.float32)
    biasrow = np.ascontiguousarray(b).reshape(1, NOUT)
    biascol = np.ascontiguousarray(b.reshape(K_FACTORS, GCN_DIM).T)
    ones = np.ones((1, INIT_DIM), dtype=np.float32)
    in_maps = []
    for m in range(N_CORES):
        pad = np.zeros((PAD_ROWS, INIT_DIM), dtype=np.float32)
        pad[:ROWS] = init_embed[m * ROWS : (m + 1) * ROWS]
        shard = np.ascontiguousarray(
            pad.reshape(N_SLABS, SLAB, INIT_DIM).transpose(0, 2, 1)
        )
        in_maps.append(
            {
                "embB": shard,
                "subT": np.ascontiguousarray(
                    gathered_T[:, m * SUB_ROWS : (m + 1) * SUB_ROWS]
                ),
                "w": w,
                "biasrow": biasrow,
                "biascol": biascol,
                "ones": ones,
            }
        )
    return in_maps


def _assemble(results, init_rel, rel):
    x = np.empty((N_ENT, NOUT), dtype=np.float32)
    for m in range(N_CORES):
        xP = results[m]["xP_out"]  # [N_SLABS, 128, 4, SLAB]
        xm = xP.transpose(0, 3, 2, 1).reshape(PAD_ROWS, NOUT)
        x[m * ROWS : (m + 1) * ROWS] = xm[:ROWS]
    x = x.reshape(N_ENT, K_FACTORS, GCN_DIM)
    sub_emb = np.concatenate(
        [results[m]["sub_out"] for m in range(N_CORES)], axis=0
    )
    gram = np.zeros((K_FACTORS, GCN_DIM, GCN_DIM), dtype=np.float64)
    for m in range(N_CORES):
        gram += results[m]["gram_out"].reshape(K_FACTORS, GCN_DIM, GCN_DIM)

    init_rel = np.asarray(init_rel, dtype=np.float32)
    rel = np.asarray(rel)
    rel_emb = np.tile(init_rel[rel], (1, K_FACTORS))

    n = GCN_DIM
    hH = np.eye(n) - np.ones((n, n)) / n
    M = np.einsum("de,kef->kdf", hH, gram)
    G = np.einsum("idf,jfd->ij", M, M)
    mi_loss = np.float32((G.sum() - np.trace(G)) / 2.0)
    return sub_emb, rel_emb, x, mi_loss


def _run(inputs, trace=False):
    from concourse.bass_utils import run_bass_kernel_spmd

    nc = _get_program()
    in_maps = _prep_in_maps(
        inputs["init_embed"], inputs["W_pca"], inputs["b_pca"], inputs["sub"]
    )
    if trace:
        _install_ntff_hook()
    res = run_bass_kernel_spmd(nc, in_maps, list(range(N_CORES)), trace=trace)
    out = _assemble(res.results, inputs["init_rel"], inputs["rel"])
    return out, res


def kernel(**inputs):
    out, _ = _run(inputs, trace=False)
    return out


def _install_ntff_hook():
    """The agent image's antenv lacks axon_hooks; shim it so trace=True
    can capture an NTFF profile (used by test.py only)."""
    import types

    if "antenv.axon_hooks" in sys.modules:
        return
    import antenv
    from trn_agent_boot.trn_boot import _ntff_profile_via_ctypes

    mod = types.ModuleType("antenv.axon_hooks")
    mod._hook = _ntff_profile_via_ctypes("/opt/axon/libaxon_pjrt.so")
    mod.set_axon_ntff_profile_hook = lambda h: setattr(mod, "_hook", h)
    mod.get_axon_ntff_profile_hook = lambda: mod._hook
    sys.modules["antenv.axon_hooks"] = mod
    antenv.axon_hooks = mod
